# revision 18
# baseline (speedup 1.0000x reference)
"""Causal multi-head attention (B=4, T=2048, D=1024, H=16) on 8 NeuronCores.

Sharding:
  stage 1 (QKV proj + attention): core c -> batch c//2, head-group c%2
    (8 of 16 heads, 512 of 1024 channels). Data-parallel on B, tensor-
    parallel on heads.
  stage 2 (output projection): one 8-rank AllToAll re-shards attention
    output to (all 4 batches x 256-token t-slice) per core, then each core
    computes out = attn_out @ W_O.T for its 1024 rows. No reduction needed.

All heavy matmuls run in fp32r (full PE rate, ~19-bit mantissa). exp runs on
the scalar engine reading PSUM directly with the softmax scale fused; the
softmax denominator comes for free as a 65th output row of the PV matmul
(V augmented with a ones column). Causal masking multiplies diagonal-block
probabilities by precomputed 0/1 masks.

The t-chunk loop interleaves projections with attention: after projecting
chunk tc, all k-tiles needed by q-chunk tc exist, so attention for q-chunk tc
runs while the next chunk's projections stream — keeping PE busy during the
ACT-heavy attention phase.
"""
import numpy as np

import concourse.bass as bass
import concourse.mybir as mybir
import concourse.tile as tile
from concourse.bass_utils import run_bass_kernel_spmd

F32 = mybir.dt.float32
F32R = mybir.dt.float32r

P = 128
B, T, D = 4, 2048, 1024
H, HD = 16, 64
NCORES = 8
CH = D // 2          # channels per core (8 heads)
NHP = 4              # head pairs per core
NKT = T // P         # 16 k-tiles
NQC = T // 512       # 4 q-chunks
NIT = D // P         # 8 input-dim tiles
TS256 = 256          # t-slice per core per batch in stage 2


def _split_multiwaits(nc) -> int:
    """walrus here rejects >1 sem wait per instruction; split extras into
    wait-only NoOps on the same engine."""
    nsplit = 0
    for f in nc.m.functions:
        for bb in f.blocks:
            if not any(
                i.sync_info is not None and i.sync_info.on_wait is not None
                and len(i.sync_info.on_wait) > 1 for i in bb.instructions
            ):
                continue
            new_list = []
            for inst in bb.instructions:
                si = inst.sync_info
                if si is not None and si.on_wait is not None and len(si.on_wait) > 1:
                    waits = list(si.on_wait)
                    for k, w in enumerate(waits[:-1]):
                        n = mybir.InstNoOp(
                            name=f"{inst.name}-wsplit{k}", ins=[], outs=[])
                        n.engine = inst.engine
                        n.sync_info = mybir.SyncInfo(on_wait=[w], on_update=[])
                        new_list.append(n)
                        nsplit += 1
                    inst.sync_info = mybir.SyncInfo(
                        on_wait=[waits[-1]], on_update=list(si.on_update or []))
                new_list.append(inst)
            bb.instructions = new_list
    return nsplit


def _build_nc(sim: bool = False, mask_mode: str = "gp"):
    nc = bass.Bass("TRN2", target_bir_lowering=False, debug=False,
                   num_devices=NCORES)
    xt_d = nc.dram_tensor("xt", [D, T], F32R, kind="ExternalInput").ap()
    wq_d = nc.dram_tensor("wq", [D, CH], F32R, kind="ExternalInput").ap()
    wk_d = nc.dram_tensor("wk", [D, CH], F32R, kind="ExternalInput").ap()
    wv_d = nc.dram_tensor("wv", [D, CH], F32R, kind="ExternalInput").ap()
    wo_d = nc.dram_tensor("wo", [D, D], F32R, kind="ExternalInput").ap()
    ones_d = nc.dram_tensor("ones", [P, NKT * NHP * 2], F32R,
                            kind="ExternalInput").ap()
    out_d = nc.dram_tensor("out", [B, 2, P, D], F32, kind="ExternalOutput").ap()
    a2a_in0 = nc.dram_tensor("a2a_in0", [NCORES, CH, P], F32R).ap()
    a2a_out0 = nc.dram_tensor("a2a_out0", [NCORES, CH, P], F32R).ap()
    a2a_in1 = nc.dram_tensor("a2a_in1", [NCORES, CH, P], F32R).ap()
    a2a_out1 = nc.dram_tensor("a2a_out1", [NCORES, CH, P], F32R).ap()

    scale = float(1.0 / np.sqrt(HD))

    with tile.TileContext(nc) as tc:
        with (
            tc.tile_pool(name="persist", bufs=1) as persist,
        ):
            # ---- persistent SBUF tensors -------------------------------
            kt_s = persist.tile([P, NHP, T], F32R)    # K^T  (channels, k)
            va = persist.tile([P, NKT, NHP, 2, HD + 1], F32R)  # V | ones
            nc.sync.dma_start(va[:, :, :, :, HD],
                              ones_d.rearrange("p (n h t) -> p n h t",
                                               n=NKT, h=NHP))

            with (
                tc.tile_pool(name="wpool", bufs=1) as wpool,
                tc.tile_pool(name="xpool", bufs=2) as xpool,
                tc.tile_pool(name="qpool", bufs=2) as qpool,
                tc.tile_pool(name="ao_pool", bufs=2) as ao_pool,
                tc.tile_pool(name="mpool", bufs=1) as mpool,
                tc.tile_pool(name="pt_pool", bufs=3) as pt_pool,
                tc.tile_pool(name="nrm_pool", bufs=1) as nrm_pool,
                tc.tile_pool(name="ppool", bufs=2, space="PSUM") as ppool,
                tc.tile_pool(name="ps_s", bufs=2, space="PSUM") as ps_s,
                tc.tile_pool(name="ps_pv", bufs=1, space="PSUM") as ps_pv,
            ):
                wq = wpool.tile([P, NIT, CH], F32R)
                wk = wpool.tile([P, NIT, CH], F32R)
                wv = wpool.tile([P, NIT, CH], F32R)
                xt_r = xt_d.rearrange("(i p) t -> p i t", p=P)
                xtc0 = xpool.tile([P, NIT, 512], F32R, tag="xtc")
                for it in range(NIT):
                    nc.sync.dma_start(xtc0[:, it], xt_r[:, it, 0:512])
                    nc.sync.dma_start(wq[:, it], wq_d.rearrange(
                        "(i p) o -> p i o", p=P)[:, it])
                    nc.sync.dma_start(wk[:, it], wk_d.rearrange(
                        "(i p) o -> p i o", p=P)[:, it])
                for it in range(NIT):
                    nc.sync.dma_start(wv[:, it], wv_d.rearrange(
                        "(i p) o -> p i o", p=P)[:, it])

                ones64 = mpool.tile([P, 64], F32R)
                nc.sync.dma_start(ones64[:], ones_d[:, 0:64])
                masks = []
                if mask_mode == "dve":
                    for i in range(4):
                        m = mpool.tile([P, 2, 512], mybir.dt.bfloat16,
                                       tag=f"mask{i}")
                        nc.gpsimd.memset(m[:], 1.0)
                        nc.gpsimd.affine_select(
                            out=m[:], in_=m[:],
                            compare_op=mybir.AluOpType.is_ge,
                            fill=0.0, base=-P * i, channel_multiplier=-1,
                            pattern=[[0, 2], [1, 512]])
                        masks.append(m)

                # pending projection psum-groups of the NEXT chunk, emitted
                # as PE filler work inside the attention kt loops
                pending = []
                pt_state = {"n": 0}

                def emit_fillers(remaining_units):
                    if not pending:
                        return
                    n = max(1, -(-len(pending) // max(1, remaining_units)))
                    for _ in range(min(n, len(pending))):
                        pending.pop(0)()

                def project(tc4, xtc=None):
                    """Queue QKV projection psum-groups for t-chunk tc4.
                    Returns the Q^T chunk tile; the groups themselves are
                    emitted later as PE filler inside attention."""
                    if xtc is None:
                        xtc = xpool.tile([P, NIT, 512], F32R, tag="xtc")
                        for it in range(NIT):
                            nc.sync.dma_start(
                                xtc[:, it],
                                xt_r[:, it, tc4 * 512:(tc4 + 1) * 512])
                    qtc = qpool.tile([P, NHP, 512], F32R, tag="qtc")

                    def qk_group(w, dst, dsl, ot):
                        def g():
                            ps = ppool.tile([P, 512], F32, tag="proj")
                            for it in range(NIT):
                                nc.tensor.matmul(
                                    ps[:], w[:, it, ot * P:(ot + 1) * P],
                                    xtc[:, it], start=(it == 0),
                                    stop=(it == NIT - 1))
                            nc.vector.tensor_copy(dst[:, ot, dsl], ps[:])
                        return g

                    def v_group(tt4):
                        def g():
                            ps = ppool.tile([P, 512], F32, tag="proj")
                            for it in range(NIT):
                                nc.tensor.matmul(
                                    ps[:], xtc[:, it, tt4 * P:(tt4 + 1) * P],
                                    wv[:, it], start=(it == 0),
                                    stop=(it == NIT - 1))
                            nc.vector.tensor_copy(
                                va[:, tc4 * 4 + tt4, :, :, 0:HD],
                                ps[:].rearrange("p (hp h d) -> p hp h d",
                                                hp=NHP, h=2))
                        return g

                    for ot in range(NHP):
                        pending.append(qk_group(wq, qtc, slice(0, 512), ot))
                    for ot in range(NHP):
                        pending.append(qk_group(
                            wk, kt_s, slice(tc4 * 512, (tc4 + 1) * 512), ot))
                    for tt4 in range(4):
                        pending.append(v_group(tt4))
                    return qtc

                def attend(hp, qc, qtc, aoq):
                    """Attention for head-pair hp, q-chunk qc. kt loop is
                    software-pipelined: QK(kt+1) issues before PV(kt) so PE
                    isn't stalled behind the exp of the current tile."""
                    nkt = 4 * (qc + 1)
                    pva = ps_pv.tile([HD + 1, 512], F32, tag="pva")
                    pvb = ps_pv.tile([HD + 1, 512], F32, tag="pvb")
                    s2s = {}
                    pts = {}

                    def qk(kt):
                        ksl = slice(kt * P, (kt + 1) * P)
                        s2 = ps_s.tile([P, 1024], F32, tag="s2")
                        nc.tensor.matmul(s2[:, 0:512], kt_s[0:64, hp, ksl],
                                         qtc[0:64, hp], start=True, stop=True)
                        nc.tensor.matmul(s2[:, 512:1024], kt_s[64:128, hp, ksl],
                                         qtc[64:128, hp], start=True, stop=True)
                        s2s[kt] = s2

                    def softmax_pv(kt, remaining):
                        s2 = s2s.pop(kt)
                        pt = pt_pool.tile([P, 2, 512], F32R, tag="pt")
                        di = kt - 4 * qc
                        if mask_mode == "gp":
                            f0 = max(0, di) * P
                            s2v = s2[:].rearrange("p (a b) -> p a b", a=2)
                            nc.scalar.activation(
                                pt[:, :, f0:], s2v[:, :, f0:],
                                mybir.ActivationFunctionType.Exp, scale=scale)
                            if di >= 0:
                                # causal: keep q >= k, zero the rest (incl the
                                # [0:f0) region the restricted exp skipped)
                                nc.gpsimd.affine_select(
                                    out=pt[:], in_=pt[:],
                                    compare_op=mybir.AluOpType.is_ge,
                                    fill=0.0, base=-P * di,
                                    channel_multiplier=-1,
                                    pattern=[[0, 2], [1, 512]])
                        else:
                            # restricted exp on diagonal blocks once every pt
                            # slot has been fully written at least once (the
                            # skipped [0:f0) region must hold finite stale
                            # data for mask*stale==0 to be safe)
                            f0 = max(0, di) * P if pt_state["n"] >= 3 else 0
                            pt_state["n"] += 1
                            if f0 > 0:
                                s2v = s2[:].rearrange("p (a b) -> p a b", a=2)
                                nc.scalar.activation(
                                    pt[:, :, f0:], s2v[:, :, f0:],
                                    mybir.ActivationFunctionType.Exp,
                                    scale=scale)
                            else:
                                nc.scalar.activation(
                                    pt[:].rearrange("p a b -> p (a b)"), s2[:],
                                    mybir.ActivationFunctionType.Exp,
                                    scale=scale)
                            if di >= 0:
                                nc.vector.tensor_mul(pt[:], pt[:],
                                                     masks[di][:])
                        nc.tensor.matmul(pva[:], va[:, kt, hp, 0], pt[:, 0],
                                         start=(kt == 0), stop=(kt == nkt - 1))
                        nc.tensor.matmul(pvb[:], va[:, kt, hp, 1], pt[:, 1],
                                         start=(kt == 0), stop=(kt == nkt - 1))
                        emit_fillers(remaining)

                    qk(0)
                    for kt in range(1, nkt):
                        qk(kt)
                        softmax_pv(kt - 1, (nkt - kt) + (NHP - 1 - hp) * nkt)
                    softmax_pv(nkt - 1, 1 + (NHP - 1 - hp) * nkt)

                    # copy PV accumulators out of PSUM fast (frees banks),
                    # then normalize: out^T rows /= denominator (row 64)
                    pvs = nrm_pool.tile([P, 2, 512], F32, tag="pvs")
                    nc.vector.tensor_copy(pvs[0:65, 0], pva[:])
                    nc.vector.tensor_copy(pvs[0:65, 1], pvb[:])
                    rden = nrm_pool.tile([P, 2, 512], F32R, tag="rden")
                    with nc.allow_low_precision("f32r softmax denominators"):
                        nc.vector.reciprocal(rden[64:65, 0], pvs[64:65, 0])
                        nc.vector.reciprocal(rden[64:65, 1], pvs[64:65, 1])
                    rba = ppool.tile([64, 512], F32, tag="proj")
                    rbb = ppool.tile([64, 512], F32, tag="proj")
                    nc.tensor.matmul(rba[:], ones64[64:65, :],
                                     rden[64:65, 0], start=True, stop=True)
                    nc.tensor.matmul(rbb[:], ones64[64:65, :],
                                     rden[64:65, 1], start=True, stop=True)
                    nc.vector.tensor_mul(aoq[0:64, hp], pvs[0:64, 0], rba[:])
                    nc.vector.tensor_mul(aoq[64:128, hp], pvs[0:64, 1], rbb[:])

                # interleaved: project chunk tc, then attention q-chunk tc,
                # streaming each finished chunk into the re-shard buffers.
                # stage-2 row owner of q = m*1024 + j*128 + p is core j, so
                # the first collective can fire once q < 1024 is done.
                a2a_r = [a.rearrange("j (hp p) t -> p hp j t", p=P)
                         for a in (a2a_in0, a2a_in1)]

                def emit_collective(m):
                    cin = (a2a_in0, a2a_in1)[m]
                    cout = (a2a_out0, a2a_out1)[m]
                    if sim:
                        nc.sync.dma_start(cout, cin)
                    else:
                        nc.gpsimd.collective_compute(
                            "AllToAll", mybir.AluOpType.bypass,
                            replica_groups=[list(range(NCORES))],
                            ins=[cin], outs=[cout])

                qtc = project(0, xtc=xtc0)
                while pending:
                    pending.pop(0)()
                for tc4 in range(NQC):
                    if tc4 + 1 < NQC:
                        next_qtc = project(tc4 + 1)  # queued as fillers
                    aoq = ao_pool.tile([P, NHP, 512], F32R, tag="aoq")
                    for hp in range(NHP):
                        attend(hp, tc4, qtc, aoq)
                    while pending:
                        pending.pop(0)()
                    m, jb = tc4 // 2, (tc4 % 2) * 4
                    for hp in range(NHP):
                        nc.sync.dma_start(
                            a2a_r[m][:, hp, jb:jb + 4],
                            aoq[:, hp].rearrange("p (j t) -> p j t", j=4))
                    if tc4 == 1:
                        emit_collective(0)
                    if tc4 + 1 < NQC:
                        qtc = next_qtc
                emit_collective(1)

            # ---- stage 2: output projection over re-sharded rows -------
            with (
                tc.tile_pool(name="wo_pool", bufs=1) as wo_pool,
                tc.tile_pool(name="aob_pool", bufs=2) as aob_pool,
                tc.tile_pool(name="o_pool", bufs=3) as o_pool,
                tc.tile_pool(name="ps_o", bufs=4, space="PSUM") as ps_o,
            ):
                wo = wo_pool.tile([P, NIT, D], F32R)
                for it in range(NIT):
                    nc.sync.dma_start(wo[:, it], wo_d.rearrange(
                        "(i p) o -> p i o", p=P)[:, it])
                for m in range(2):
                    cout = (a2a_out0, a2a_out1)[m]
                    for b in range(B):
                        aob = aob_pool.tile([P, NIT, P], F32R)
                        nc.sync.dma_start(
                            aob[:],
                            cout[2 * b:2 * b + 2].rearrange(
                                "s (c p) t -> p (s c) t", p=P))
                        osb = o_pool.tile([P, D], F32)
                        for oc in range(2):
                            ps = ps_o.tile([P, 512], F32, tag="opj")
                            for ct in range(NIT):
                                nc.tensor.matmul(
                                    ps[:], aob[:, ct],
                                    wo[:, ct, oc * 512:(oc + 1) * 512],
                                    start=(ct == 0), stop=(ct == NIT - 1))
                            nc.vector.tensor_copy(
                                osb[:, oc * 512:(oc + 1) * 512], ps[:])
                        nc.sync.dma_start(out_d[b, m], osb[:])
    _split_multiwaits(nc)
    return nc


_NC_CACHE = None


def _get_nc():
    global _NC_CACHE
    if _NC_CACHE is None:
        _NC_CACHE = _build_nc()
    return _NC_CACHE


def make_in_maps(x, W_Q, W_K, W_V, W_O):
    wqt = np.ascontiguousarray(W_Q.T)
    wkt = np.ascontiguousarray(W_K.T)
    wvt = np.ascontiguousarray(W_V.T)
    wot = np.ascontiguousarray(W_O.T)
    ones = np.ones((P, NKT * NHP * 2), np.float32)
    in_maps = []
    for c in range(NCORES):
        b, g = c // 2, c % 2
        in_maps.append({
            "xt": np.ascontiguousarray(x[b].T),
            "wq": np.ascontiguousarray(wqt[:, g * CH:(g + 1) * CH]),
            "wk": np.ascontiguousarray(wkt[:, g * CH:(g + 1) * CH]),
            "wv": np.ascontiguousarray(wvt[:, g * CH:(g + 1) * CH]),
            "wo": wot,
            "ones": ones,
        })
    return in_maps


def assemble(results):
    out = np.empty((B, T, D), np.float32)
    for j in range(NCORES):
        o = results[j]["out"]  # [B, 2, 128, D]
        for b in range(B):
            for m in range(2):
                r0 = m * 1024 + j * P
                out[b, r0:r0 + P, :] = o[b, m]
    return out


def kernel(x, W_Q, W_K, W_V, W_O):
    x = np.asarray(x, np.float32)
    in_maps = make_in_maps(x, np.asarray(W_Q, np.float32),
                           np.asarray(W_K, np.float32),
                           np.asarray(W_V, np.float32),
                           np.asarray(W_O, np.float32))
    nc = _get_nc()
    res = run_bass_kernel_spmd(nc, in_maps, core_ids=list(range(NCORES)))
    return assemble(res.results)
